# revision 49
# baseline (speedup 1.0000x reference)
"""nn_GT_7327214207519 — 2-layer TransformerConv GNN (heads=4) on 8 trn2 NeuronCores.

Strategy: destination-sharded edge phase (2500 dsts/core in 20 tiles of 125),
edges pre-sorted by dst on host so per-dst softmax reduces entirely on-chip via
one-hot matmuls in PSUM — no segment-max pass (logits are tiny, softmax is
shift-invariant) and no cross-core reductions. L0 projections are replicated
(x is replicated), L1 projections are sharded from the local h shard and
exchanged with a single AllGather of the concatenated [k1|v1] bf16 table.
"""
import contextlib
import ctypes
import sys
import types

import numpy as np
import ml_dtypes

BF16 = ml_dtypes.bfloat16

N, E, D_IN, HID, OUT, H = 20000, 320000, 128, 128, 128, 4
HC = H * HID            # 512
NCORES = 8
NPC = N // NCORES       # 2500 nodes per core
TD = 125                # dsts per tile
NT = NPC // TD          # 20 tiles per core
SCALE = 1.0 / np.sqrt(np.float32(HID))
AGC = 4                 # AllGather chunks
GRP = 2                 # gather groups per tile (AGC/GRP chunks each)
CR = NPC // AGC         # rows per AG chunk (per core)


def _perm_row(g):
    """Gather-table row permutation [chunk][core][row] that makes each
    AllGather chunk's output a contiguous slice. Works on arrays."""
    l = g % NPC
    return (l // CR) * (CR * NCORES) + (g // NPC) * CR + (l % CR)

TRACE = False           # set by test harness to capture an NTFF profile
LAST_EXEC_NS = None     # filled when TRACE
LAST_RESULTS = None


def _install_ntff_hook(so_path="/opt/axon/libaxon_pjrt.so"):
    """Register antenv.axon_hooks with the ctypes NTFF profile hook."""
    if "antenv.axon_hooks" in sys.modules:
        return
    try:
        lib = ctypes.CDLL(so_path)
    except OSError:
        return
    if not hasattr(lib, "axon_start_nrt_profile"):
        return
    lib.axon_start_nrt_profile.argtypes = [ctypes.POINTER(ctypes.c_int64), ctypes.c_size_t]
    lib.axon_start_nrt_profile.restype = ctypes.c_int64
    lib.axon_stop_nrt_profile.argtypes = [ctypes.c_char_p]
    lib.axon_stop_nrt_profile.restype = ctypes.c_int64

    @contextlib.contextmanager
    def _hook(output_dir, device_ids):
        import jax
        jax.devices()
        if device_ids:
            ids = (ctypes.c_int64 * len(device_ids))(*device_ids)
            rc = lib.axon_start_nrt_profile(ids, len(device_ids))
        else:
            rc = lib.axon_start_nrt_profile(None, 0)
        if rc != 0:
            raise RuntimeError(f"axon_start_nrt_profile rc={rc}")
        try:
            yield
        finally:
            n = lib.axon_stop_nrt_profile(str(output_dir).encode())
            print(f"profile: {n} file(s) written to {output_dir}", file=sys.stderr)

    mod = types.ModuleType("antenv.axon_hooks")
    mod.get_axon_ntff_profile_hook = lambda: _hook
    mod.set_axon_ntff_profile_hook = lambda h: None
    sys.modules["antenv.axon_hooks"] = mod


_NC_CACHE = {}


def _build_nc(B):
    """Build the SPMD Bass program for B edge-blocks per dst-tile."""
    from concourse import bass, mybir, tile, bacc
    from concourse.masks import make_identity

    GS = B * 128              # slots per gather group
    SLOTS = GRP * GS          # slots per tile
    CRN = CR * NCORES         # table rows per AG chunk
    GRN = N // GRP            # table rows per gather group
    f32, bf16, i16, i32 = (mybir.dt.float32, mybir.dt.bfloat16,
                           mybir.dt.int16, mybir.dt.int32)
    fp8 = mybir.dt.float8e4

    nc = bacc.Bacc("TRN2", target_bir_lowering=False, debug=False,
                   num_devices=NCORES)

    # ---- inputs (per-core values differ only for edge arrays / xT_own) ----
    xT = nc.dram_tensor("xT", [D_IN, N], bf16, kind="ExternalInput").ap()
    xTo = nc.dram_tensor("xTo", [D_IN, NPC], bf16, kind="ExternalInput").ap()
    srcw = nc.dram_tensor("srcw", [NT, GRP, 128, B * 8], i16, kind="ExternalInput").ap()
    mw = nc.dram_tensor("mw", [NT, 128, SLOTS], fp8, kind="ExternalInput").ap()
    mtw = nc.dram_tensor("mtw", [NT, TD, SLOTS], fp8, kind="ExternalInput").ap()

    w0 = {}
    for nm in ("Wq0", "Wk0", "Wv0", "Ws0"):
        w0[nm] = nc.dram_tensor(nm, [D_IN, HC], bf16, kind="ExternalInput").ap()
    w1 = {}
    for nm in ("Wq1", "Wk1", "Wv1"):
        w1[nm] = nc.dram_tensor(nm, [HC, HC], bf16, kind="ExternalInput").ap()
    w1["Ws1"] = nc.dram_tensor("Ws1", [HC, OUT], bf16, kind="ExternalInput").ap()
    bias = {}
    for nm in ("bq0", "bs0", "bq1"):
        bias[nm] = nc.dram_tensor(nm, [1, HC], bf16, kind="ExternalInput").ap()
    bias["bs1"] = nc.dram_tensor("bs1", [1, OUT], bf16, kind="ExternalInput").ap()

    out = nc.dram_tensor("out", [NPC, OUT], f32, kind="ExternalOutput").ap()

    # ---- internal DRAM ----
    kv0_b = nc.dram_tensor("kv0_b", [NPC, 2 * HC], bf16)
    kv0_full = nc.dram_tensor("kv0_full", [N, 2 * HC], bf16, addr_space="Shared")
    q0_loc = nc.dram_tensor("q0_loc", [NPC, HC], bf16)
    s0_own = nc.dram_tensor("s0_own", [NPC, HC], f32)
    q1_loc = nc.dram_tensor("q1_loc", [NPC, HC], bf16)
    s1_own = nc.dram_tensor("s1_own", [NPC, OUT], f32)
    kv1_b = nc.dram_tensor("kv1_b", [NPC, 2 * HC], bf16)
    kv1_full = nc.dram_tensor("kv1_full", [N, 2 * HC], bf16, addr_space="Shared")

    with tile.TileContext(nc) as tc, contextlib.ExitStack() as ctx:
        cst = ctx.enter_context(tc.tile_pool(name="cst", bufs=1))
        prj = ctx.enter_context(tc.tile_pool(name="prj", bufs=3))
        edg = ctx.enter_context(tc.tile_pool(name="edg", bufs=4))
        sml = ctx.enter_context(tc.tile_pool(name="sml", bufs=4))
        psA = ctx.enter_context(tc.tile_pool(name="psA", bufs=3, space="PSUM"))
        psD = ctx.enter_context(tc.tile_pool(name="psD", bufs=1, space="PSUM"))
        psT = ctx.enter_context(tc.tile_pool(name="psT", bufs=1, space="PSUM"))
        psP = ctx.enter_context(tc.tile_pool(name="psP", bufs=3, space="PSUM"))

        # ---- constants in SBUF ----
        w0_sb = {}
        for nm in ("Wq0", "Wk0", "Wv0", "Ws0"):
            t = cst.tile([D_IN, HC], bf16, tag=nm)
            nc.sync.dma_start(out=t[:], in_=w0[nm])
            w0_sb[nm] = t
        w1_sb = {}
        for nm in ("Wq1", "Wk1", "Wv1"):
            t = cst.tile([128, 4, HC], bf16, tag=nm)
            nc.sync.dma_start(out=t[:], in_=w1[nm].rearrange("(k p) n -> p k n", p=128))
            w1_sb[nm] = t
        ws1_sb = cst.tile([128, 4, OUT], bf16, tag="Ws1")
        nc.sync.dma_start(out=ws1_sb[:], in_=w1["Ws1"].rearrange("(k p) n -> p k n", p=128))
        b_sb = {}
        for nm, width in (("bq0", HC), ("bs0", HC),
                          ("bq1", HC), ("bs1", OUT)):
            t = cst.tile([1, width], bf16, tag=nm)
            nc.sync.dma_start(out=t[:], in_=bias[nm])
            b_sb[nm] = t
        ones = cst.tile([1, 128], bf16, tag="ones")
        nc.gpsimd.memset(ones[:], 1.0)
        ident = cst.tile([128, 128], bf16, tag="ident")
        make_identity(nc, ident[:])


        def proj_tile(xt, W_sb, b_row, m, psum_pool):
            """psum[m, width] = xt.T @ W (+ 1s*b when b_row given).
            k needs no bias (softmax shift-invariance per destination); the v
            bias is folded into the skip table's bias on the host."""
            last = b_row is None
            if len(W_sb.shape) == 3:
                width = W_sb.shape[2]
                ps = psum_pool.tile([TD, width], f32, tag="pp")
                for kk in range(4):
                    nc.tensor.matmul(ps[:m], lhsT=xt[:, kk, :m], rhs=W_sb[:, kk, :],
                                     start=(kk == 0), stop=(last and kk == 3))
            else:
                width = W_sb.shape[1]
                ps = psum_pool.tile([TD, width], f32, tag="pp")
                nc.tensor.matmul(ps[:m], lhsT=xt[:, :m], rhs=W_sb[:], start=True, stop=last)
            if b_row is not None:
                nc.tensor.matmul(ps[:m], lhsT=ones[:, :m], rhs=b_row[:], start=False, stop=True)
            return ps

        # ---- phase P0: L0 projections (kv0 sharded + chunked AllGather) ----
        for rt in range(NT):
            sl = slice(rt * TD, (rt + 1) * TD)
            xt = prj.tile([D_IN, TD], bf16, tag="xt")
            nc.scalar.dma_start(out=xt[:], in_=xTo[:, sl])
            ps_k = proj_tile(xt, w0_sb["Wk0"], None, TD, psP)
            kb = prj.tile([TD, HC], bf16, tag="kb")
            nc.scalar.copy(out=kb[:], in_=ps_k[:])
            nc.sync.dma_start(out=kv0_b[sl, :HC], in_=kb[:])
            ps_v = proj_tile(xt, w0_sb["Wv0"], None, TD, psP)
            vb = prj.tile([TD, HC], bf16, tag="vb")
            nc.scalar.copy(out=vb[:], in_=ps_v[:])
            nc.sync.dma_start(out=kv0_b[sl, HC:], in_=vb[:])
            ps_q = proj_tile(xt, w0_sb["Wq0"], b_sb["bq0"], TD, psP)
            qb = prj.tile([TD, HC], bf16, tag="qb")
            nc.scalar.copy(out=qb[:], in_=ps_q[:])
            nc.sync.dma_start(out=q0_loc[sl, :], in_=qb[:])
            ps_s = proj_tile(xt, w0_sb["Ws0"], b_sb["bs0"], TD, psP)
            sb_ = prj.tile([TD, HC], f32, tag="sb")
            nc.scalar.copy(out=sb_[:], in_=ps_s[:])
            nc.sync.dma_start(out=s0_own[sl, :], in_=sb_[:])
        for c in range(AGC):
            nc.gpsimd.collective_compute(
                "AllGather", mybir.AluOpType.bypass,
                replica_groups=[list(range(NCORES))],
                ins=[kv0_b.ap()[c * CR:(c + 1) * CR, :].opt()],
                outs=[kv0_full.ap()[c * CRN:(c + 1) * CRN, :].opt()])

        def edge_tile(t, q_tab, kv_tab):
            """Group-pipelined gather + logits + masked aggregation for one
            dst-tile; gather group c reads only AG chunk c of the table.
            Returns (psum_agg [TD, HC] f32, psum_den [TD, H] f32)."""
            q_tile = sml.tile([TD, HC], bf16, tag="qtl")
            nc.sync.dma_start(out=q_tile[:], in_=q_tab[t * TD:(t + 1) * TD, :])

            lg = sml.tile([128, GRP * B * H], f32, tag="lg")
            wexp = sml.tile([128, GRP * B * H], bf16, tag="wexp")
            ps_agg = psA.tile([TD, HC], f32, tag="agg")
            ps_den = psD.tile([TD, H], f32, tag="den")

            for c in range(GRP):
                src_t = sml.tile([128, B * 8], i16, tag="src")
                nc.sync.dma_start(out=src_t[:], in_=srcw[t, c])
                kvc = edg.tile([128, B, 2 * HC], bf16, tag="kv")
                for g0 in range(0, B, 8):
                    g1 = min(g0 + 8, B)
                    nc.gpsimd.dma_gather(
                        out_ap=kvc[:, g0:g1, :],
                        in_ap=kv_tab[c * GRN:(c + 1) * GRN, :],
                        idxs_ap=src_t[:, g0 * 8:g1 * 8],
                        num_idxs=(g1 - g0) * 128, num_idxs_reg=(g1 - g0) * 128,
                        elem_size=2 * HC, elem_step=2 * HC)
                sb0 = c * GS
                moh = edg.tile([128, B, 128], fp8, tag="moh")
                nc.sync.dma_start(
                    out=moh[:],
                    in_=mw[t, :, sb0:sb0 + GS].rearrange("p (b d) -> p b d", d=128))
                mtc = edg.tile([TD, GS], fp8, tag="mt")
                nc.scalar.dma_start(out=mtc[:], in_=mtw[t, :, sb0:sb0 + GS])
                qkc = edg.tile([128, B, HC], bf16, tag="qkt")
                for b in range(B):
                    ps_qg = psP.tile([128, 512], f32, tag="pp")
                    nc.tensor.matmul(ps_qg[:, :HC], lhsT=mtc[:, b * 128:(b + 1) * 128],
                                     rhs=q_tile[:], start=True, stop=True)
                    nc.scalar.copy(out=qkc[:, b, :], in_=ps_qg[:, :HC])
                nc.vector.tensor_tensor(out=qkc[:], in0=qkc[:], in1=kvc[:, :, :HC],
                                        op=mybir.AluOpType.mult)
                lgs = lg[:, (c * B) * H:(c * B + B) * H]
                nc.vector.tensor_reduce(
                    out=lgs, in_=qkc[:].rearrange("p b (h c) -> p (b h) c", c=HID),
                    axis=mybir.AxisListType.X, op=mybir.AluOpType.add)
                wes = wexp[:, (c * B) * H:(c * B + B) * H]
                nc.scalar.activation(out=wes, in_=lgs,
                                     func=mybir.ActivationFunctionType.Exp,
                                     scale=float(SCALE))
                vview = kvc[:, :, HC:].rearrange("p b (h c) -> p b h c", c=HID)
                wview = wes.rearrange("p (b h) -> p b h", h=H)
                nc.vector.tensor_tensor(
                    out=vview, in0=vview,
                    in1=wview[:, :, :, None].to_broadcast([128, B, H, HID]),
                    op=mybir.AluOpType.mult)
                for b in range(B):
                    nc.tensor.matmul(ps_agg[:], lhsT=moh[:, b, :TD],
                                     rhs=kvc[:, b, HC:],
                                     start=(c == 0 and b == 0),
                                     stop=(c == GRP - 1 and b == B - 1))
                    nc.tensor.matmul(ps_den[:], lhsT=moh[:, b, :TD],
                                     rhs=wexp[:, (c * B + b) * H:(c * B + b + 1) * H],
                                     start=(c == 0 and b == 0),
                                     stop=(c == GRP - 1 and b == B - 1))
            return ps_agg, ps_den

        # ---- phase P1: L0 edge phase + per-tile L1 projections ----
        for t in range(NT):
            sl = slice(t * TD, (t + 1) * TD)
            ps_agg, ps_den = edge_tile(t, q0_loc.ap(), kv0_full.ap())
            den = sml.tile([TD, H], f32, tag="den_s")
            nc.vector.tensor_scalar(out=den[:], in0=ps_den[:], scalar1=1e-16,
                                    scalar2=None, op0=mybir.AluOpType.add)
            rec = sml.tile([TD, H], f32, tag="rec")
            nc.vector.reciprocal(out=rec[:], in_=den[:])
            s0_t = sml.tile([TD, HC], f32, tag="s0t")
            nc.sync.dma_start(out=s0_t[:], in_=s0_own[sl, :])
            hn = sml.tile([TD, HC], f32, tag="hn")
            nc.vector.tensor_tensor(
                out=hn[:].rearrange("p (h c) -> p h c", h=H),
                in0=ps_agg[:].rearrange("p (h c) -> p h c", h=H),
                in1=rec[:, :, None].to_broadcast([TD, H, HID]),
                op=mybir.AluOpType.mult)
            nc.vector.tensor_tensor(out=hn[:], in0=hn[:], in1=s0_t[:],
                                    op=mybir.AluOpType.add)
            h_bf = sml.tile([TD, HC], bf16, tag="hbf")
            nc.scalar.activation(out=h_bf[:], in_=hn[:],
                                 func=mybir.ActivationFunctionType.Relu)

            # transpose h tile -> hT [128, 4, TD]
            hT = sml.tile([128, 4, TD], bf16, tag="hT")
            for kk in range(4):
                ps_t = psT.tile([128, TD], bf16, tag="tr")
                nc.tensor.transpose(ps_t[:], in_=h_bf[:, kk * 128:(kk + 1) * 128],
                                    identity=ident[:TD, :TD])
                nc.scalar.copy(out=hT[:, kk, :], in_=ps_t[:])

            # L1 projections for this tile
            ps_q1 = proj_tile(hT, w1_sb["Wq1"], b_sb["bq1"], TD, psP)
            q1b = prj.tile([TD, HC], bf16, tag="q1b")
            nc.scalar.copy(out=q1b[:], in_=ps_q1[:])
            nc.sync.dma_start(out=q1_loc[sl, :], in_=q1b[:])
            ps_k1 = proj_tile(hT, w1_sb["Wk1"], None, TD, psP)
            k1b = prj.tile([TD, HC], bf16, tag="k1b")
            nc.scalar.copy(out=k1b[:], in_=ps_k1[:])
            nc.sync.dma_start(out=kv1_b[sl, :HC], in_=k1b[:])
            ps_v1 = proj_tile(hT, w1_sb["Wv1"], None, TD, psP)
            v1b = prj.tile([TD, HC], bf16, tag="v1b")
            nc.scalar.copy(out=v1b[:], in_=ps_v1[:])
            nc.scalar.dma_start(out=kv1_b[sl, HC:], in_=v1b[:])
            ps_s1 = proj_tile(hT, ws1_sb, b_sb["bs1"], TD, psP)
            s1b = prj.tile([TD, OUT], f32, tag="s1b")
            nc.scalar.copy(out=s1b[:], in_=ps_s1[:])
            nc.sync.dma_start(out=s1_own[sl, :], in_=s1b[:])

        # ---- AllGather [k1|v1], chunked for overlap with the P1 tail ----
        # Table rows are permuted [chunk][core][row] (see _perm_rows) so each
        # chunk's AllGather output is a contiguous slice.
        rows = NPC // AGC
        for c in range(AGC):
            nc.gpsimd.collective_compute(
                "AllGather", mybir.AluOpType.bypass,
                replica_groups=[list(range(NCORES))],
                ins=[kv1_b.ap()[c * rows:(c + 1) * rows, :].opt()],
                outs=[kv1_full.ap()[c * rows * NCORES:(c + 1) * rows * NCORES, :].opt()])

        # ---- phase P2: L1 edge phase ----
        for t in range(NT):
            sl = slice(t * TD, (t + 1) * TD)
            ps_agg, ps_den = edge_tile(t, q1_loc.ap(), kv1_full.ap())
            den = sml.tile([TD, H], f32, tag="den_s")
            nc.vector.tensor_scalar(out=den[:], in0=ps_den[:], scalar1=float(H),
                                    scalar2=1e-16, op0=mybir.AluOpType.mult,
                                    op1=mybir.AluOpType.add)
            rec = sml.tile([TD, H], f32, tag="rec")
            nc.vector.reciprocal(out=rec[:], in_=den[:])
            hn = sml.tile([TD, HC], f32, tag="hn")
            nc.vector.tensor_tensor(
                out=hn[:].rearrange("p (h c) -> p h c", h=H),
                in0=ps_agg[:].rearrange("p (h c) -> p h c", h=H),
                in1=rec[:, :, None].to_broadcast([TD, H, HID]),
                op=mybir.AluOpType.mult)
            hm = sml.tile([TD, OUT], f32, tag="hm")
            nc.vector.tensor_reduce(out=hm[:], in_=hn[:].rearrange("p (h c) -> p c h", h=H),
                                    axis=mybir.AxisListType.X,
                                    op=mybir.AluOpType.add)
            s1_t = sml.tile([TD, OUT], f32, tag="s1t")
            nc.sync.dma_start(out=s1_t[:], in_=s1_own[sl, :])
            ot = sml.tile([TD, OUT], f32, tag="ot")
            nc.vector.tensor_tensor(out=ot[:], in0=hm[:], in1=s1_t[:],
                                    op=mybir.AluOpType.add)
            nc.sync.dma_start(out=out[sl, :], in_=ot[:])

    nc.compile()
    return nc


def _prep_edges(src, dst):
    """Sort edges by (dst-tile, src-AG-chunk); build per-(core,tile,group)
    padded slot arrays with chunk-local permuted gather indices."""
    ds = dst.astype(np.int64)
    ss = src.astype(np.int64)
    sp = _perm_row(ss)                      # permuted table row of src
    tid = ds // TD                          # global dst tile 0..159
    grp = sp // (N // GRP)                  # gather group of src
    n_tiles = NCORES * NT
    key = tid * GRP + grp
    order = np.argsort(key, kind="stable")
    key_s = key[order]
    sp_s = sp[order]
    ds_s = ds[order]
    cnt = np.bincount(key_s, minlength=n_tiles * GRP)
    B = int(np.ceil(cnt.max() / 128))
    GS = B * 128
    offs = np.zeros(n_tiles * GRP + 1, np.int64)
    np.cumsum(cnt, out=offs[1:])
    pos = np.arange(E) - offs[key_s]

    src_pad = np.zeros((n_tiles * GRP, GS), np.int16)
    dstc_pad = np.full((n_tiles * GRP, GS), -1.0, np.float32)
    src_pad[key_s, pos] = sp_s % (N // GRP)         # group-local row
    dstc_pad[key_s, pos] = (ds_s - (key_s // GRP) * TD).astype(np.float32)

    # 16-wrapped gather index layout, replicated across the 8 Q7 cores
    w = src_pad.reshape(n_tiles * GRP, GS // 16, 16).transpose(0, 2, 1)
    w = np.tile(w, (1, 8, 1))
    srcw = np.ascontiguousarray(w).reshape(NCORES, NT, GRP, 128, GS // 16)

    FP8 = ml_dtypes.float8_e4m3
    SLOTS = GRP * GS
    dtile = dstc_pad.reshape(n_tiles, SLOTS)
    dr = dtile.reshape(n_tiles, SLOTS // 128, 128)
    m1 = (dr[:, :, :, None] == np.arange(128, dtype=np.float32))
    mw = np.ascontiguousarray(m1.transpose(0, 2, 1, 3)).astype(FP8).reshape(
        NCORES, NT, 128, SLOTS)
    mtw = (dtile[:, None, :] == np.arange(TD, dtype=np.float32)[None, :, None]
           ).astype(FP8).reshape(NCORES, NT, TD, SLOTS)
    return B, srcw, mw, mtw


def kernel(x, edge_index,
           Wq0, bq0, Wk0, bk0, Wv0, bv0, Ws0, bs0,
           Wq1, bq1, Wk1, bk1, Wv1, bv1, Ws1, bs1):
    global LAST_EXEC_NS, LAST_RESULTS
    _install_ntff_hook()
    from concourse import bass_utils

    x = np.asarray(x, np.float32)
    edge_index = np.asarray(edge_index)
    B, srcw, mw, mtw = _prep_edges(edge_index[0], edge_index[1])

    if B not in _NC_CACHE:
        _NC_CACHE[B] = _build_nc(B)
    nc = _NC_CACHE[B]

    xT = np.ascontiguousarray(np.asarray(x).T).astype(BF16)
    shared = dict(xT=xT)
    for nm, a in (("Wq0", Wq0), ("Wk0", Wk0), ("Wv0", Wv0), ("Ws0", Ws0),
                  ("Wq1", Wq1), ("Wk1", Wk1), ("Wv1", Wv1), ("Ws1", Ws1)):
        shared[nm] = np.asarray(a).astype(BF16)
    # k biases are softmax-invariant (dropped); v biases fold into the skip
    # tables: h = relu(num/den + bv0 + x@Ws0 + bs0), L1 mean-over-heads of bv1.
    bs0f = np.asarray(bs0, np.float32) + np.asarray(bv0, np.float32)
    bs1f = (np.asarray(bs1, np.float32)
            + np.asarray(bv1, np.float32).reshape(H, OUT).mean(axis=0))
    for nm, a in (("bq0", bq0), ("bs0", bs0f),
                  ("bq1", bq1), ("bs1", bs1f)):
        shared[nm] = np.asarray(a).astype(BF16)[None, :]

    in_maps = []
    for c in range(NCORES):
        m = dict(shared)
        m["xTo"] = np.ascontiguousarray(xT[:, c * NPC:(c + 1) * NPC])
        m["srcw"] = srcw[c]
        m["mw"] = mw[c]
        m["mtw"] = mtw[c]
        in_maps.append(m)

    res = bass_utils.run_bass_kernel_spmd(
        nc, in_maps, core_ids=list(range(NCORES)), trace=TRACE)
    LAST_EXEC_NS = res.exec_time_ns
    LAST_RESULTS = res
    out = np.concatenate([res.results[c]["out"] for c in range(NCORES)], axis=0)
    return np.ascontiguousarray(out, dtype=np.float32)


# revision 50
# speedup vs baseline: 1.0309x; 1.0309x over previous
"""nn_GT_7327214207519 — 2-layer TransformerConv GNN (heads=4) on 8 trn2 NeuronCores.

Destination-sharded SPMD design (one NEFF on 8 cores):
- Each core owns 2500 destination nodes (20 tiles x 125 dsts). The host sorts
  edges by (dst tile, src table-half) into padded 128-edge blocks, so the
  per-destination softmax reduces entirely on-chip via one-hot matmuls
  accumulating in PSUM — no segment-max pass (softmax is shift-invariant and
  the logits are tiny) and no cross-core reductions, no scatter.
- Both layers' [k|v] tables are bf16, row-interleaved, and row-permuted
  [chunk][core][row] so chunked AllGathers land in contiguous slices; each
  tile's gathers are split into two groups that each depend on only half the
  table, letting gathers start as soon as the first AG chunks land.
- Per-edge k|v rows come from single dma_gather ops (2KB rows, <=8 blocks per
  gather to fit the 128-entry SWDGE ring). q is never gathered: a dst-major
  one-hot (host-built fp8) expands the 125 q rows to edge-major via PE matmul.
- k-bias is dropped (softmax-invariant); v-bias is folded into the skip
  tables' bias on the host; q/s biases ride as rank-1 ones x bias matmuls.
- Layer-0 q/k/v/s projections are sharded (each core projects only its own
  2500 rows from host-supplied xT) and kv0 is AllGathered in 4 chunks; layer-1
  projections reuse per-tile PE transposes of h and feed the kv1 AllGather.
"""
import contextlib
import ctypes
import sys
import types

import numpy as np
import ml_dtypes

BF16 = ml_dtypes.bfloat16

N, E, D_IN, HID, OUT, H = 20000, 320000, 128, 128, 128, 4
HC = H * HID            # 512
NCORES = 8
NPC = N // NCORES       # 2500 nodes per core
TD = 125                # dsts per tile
NT = NPC // TD          # 20 tiles per core
SCALE = 1.0 / np.sqrt(np.float32(HID))
AGC = 4                 # AllGather chunks
GRP = 2                 # gather groups per tile (AGC/GRP chunks each)
CR = NPC // AGC         # rows per AG chunk (per core)


def _perm_row(g):
    """Gather-table row permutation [chunk][core][row] that makes each
    AllGather chunk's output a contiguous slice. Works on arrays."""
    l = g % NPC
    return (l // CR) * (CR * NCORES) + (g // NPC) * CR + (l % CR)

TRACE = False           # set by test harness to capture an NTFF profile
LAST_EXEC_NS = None     # filled when TRACE
LAST_RESULTS = None


def _install_ntff_hook(so_path="/opt/axon/libaxon_pjrt.so"):
    """Register antenv.axon_hooks with the ctypes NTFF profile hook."""
    if "antenv.axon_hooks" in sys.modules:
        return
    try:
        lib = ctypes.CDLL(so_path)
    except OSError:
        return
    if not hasattr(lib, "axon_start_nrt_profile"):
        return
    lib.axon_start_nrt_profile.argtypes = [ctypes.POINTER(ctypes.c_int64), ctypes.c_size_t]
    lib.axon_start_nrt_profile.restype = ctypes.c_int64
    lib.axon_stop_nrt_profile.argtypes = [ctypes.c_char_p]
    lib.axon_stop_nrt_profile.restype = ctypes.c_int64

    @contextlib.contextmanager
    def _hook(output_dir, device_ids):
        import jax
        jax.devices()
        if device_ids:
            ids = (ctypes.c_int64 * len(device_ids))(*device_ids)
            rc = lib.axon_start_nrt_profile(ids, len(device_ids))
        else:
            rc = lib.axon_start_nrt_profile(None, 0)
        if rc != 0:
            raise RuntimeError(f"axon_start_nrt_profile rc={rc}")
        try:
            yield
        finally:
            n = lib.axon_stop_nrt_profile(str(output_dir).encode())
            print(f"profile: {n} file(s) written to {output_dir}", file=sys.stderr)

    mod = types.ModuleType("antenv.axon_hooks")
    mod.get_axon_ntff_profile_hook = lambda: _hook
    mod.set_axon_ntff_profile_hook = lambda h: None
    sys.modules["antenv.axon_hooks"] = mod


_NC_CACHE = {}


def _build_nc(B):
    """Build the SPMD Bass program for B edge-blocks per dst-tile."""
    from concourse import bass, mybir, tile, bacc
    from concourse.masks import make_identity

    GS = B * 128              # slots per gather group
    SLOTS = GRP * GS          # slots per tile
    CRN = CR * NCORES         # table rows per AG chunk
    GRN = N // GRP            # table rows per gather group
    f32, bf16, i16, i32 = (mybir.dt.float32, mybir.dt.bfloat16,
                           mybir.dt.int16, mybir.dt.int32)
    fp8 = mybir.dt.float8e4

    nc = bacc.Bacc("TRN2", target_bir_lowering=False, debug=False,
                   num_devices=NCORES)

    # ---- inputs (per-core values differ only for edge arrays / xT_own) ----
    xT = nc.dram_tensor("xT", [D_IN, N], bf16, kind="ExternalInput").ap()
    xTo = nc.dram_tensor("xTo", [D_IN, NPC], bf16, kind="ExternalInput").ap()
    srcw = nc.dram_tensor("srcw", [NT, GRP, 128, B * 8], i16, kind="ExternalInput").ap()
    mw = nc.dram_tensor("mw", [NT, 128, SLOTS], fp8, kind="ExternalInput").ap()
    mtw = nc.dram_tensor("mtw", [NT, TD, SLOTS], fp8, kind="ExternalInput").ap()

    w0 = {}
    for nm in ("Wq0", "Wk0", "Wv0", "Ws0"):
        w0[nm] = nc.dram_tensor(nm, [D_IN, HC], bf16, kind="ExternalInput").ap()
    w1 = {}
    for nm in ("Wq1", "Wk1", "Wv1"):
        w1[nm] = nc.dram_tensor(nm, [HC, HC], bf16, kind="ExternalInput").ap()
    w1["Ws1"] = nc.dram_tensor("Ws1", [HC, OUT], bf16, kind="ExternalInput").ap()
    bias = {}
    for nm in ("bq0", "bs0", "bq1"):
        bias[nm] = nc.dram_tensor(nm, [1, HC], bf16, kind="ExternalInput").ap()
    bias["bs1"] = nc.dram_tensor("bs1", [1, OUT], bf16, kind="ExternalInput").ap()

    out = nc.dram_tensor("out", [NPC, OUT], f32, kind="ExternalOutput").ap()

    # ---- internal DRAM ----
    kv0_b = nc.dram_tensor("kv0_b", [NPC, 2 * HC], bf16)
    kv0_full = nc.dram_tensor("kv0_full", [N, 2 * HC], bf16, addr_space="Shared")
    q0_loc = nc.dram_tensor("q0_loc", [NPC, HC], bf16)
    s0_own = nc.dram_tensor("s0_own", [NPC, HC], f32)
    q1_loc = nc.dram_tensor("q1_loc", [NPC, HC], bf16)
    s1_own = nc.dram_tensor("s1_own", [NPC, OUT], f32)
    kv1_b = nc.dram_tensor("kv1_b", [NPC, 2 * HC], bf16)
    kv1_full = nc.dram_tensor("kv1_full", [N, 2 * HC], bf16, addr_space="Shared")

    with tile.TileContext(nc) as tc, contextlib.ExitStack() as ctx:
        cst = ctx.enter_context(tc.tile_pool(name="cst", bufs=1))
        prj = ctx.enter_context(tc.tile_pool(name="prj", bufs=3))
        edg = ctx.enter_context(tc.tile_pool(name="edg", bufs=4))
        sml = ctx.enter_context(tc.tile_pool(name="sml", bufs=4))
        psA = ctx.enter_context(tc.tile_pool(name="psA", bufs=3, space="PSUM"))
        psD = ctx.enter_context(tc.tile_pool(name="psD", bufs=1, space="PSUM"))
        psT = ctx.enter_context(tc.tile_pool(name="psT", bufs=1, space="PSUM"))
        psP = ctx.enter_context(tc.tile_pool(name="psP", bufs=3, space="PSUM"))

        # ---- constants in SBUF ----
        w0_sb = {}
        for nm in ("Wq0", "Wk0", "Wv0", "Ws0"):
            t = cst.tile([D_IN, HC], bf16, tag=nm)
            nc.sync.dma_start(out=t[:], in_=w0[nm])
            w0_sb[nm] = t
        w1_sb = {}
        for nm in ("Wq1", "Wk1", "Wv1"):
            t = cst.tile([128, 4, HC], bf16, tag=nm)
            nc.sync.dma_start(out=t[:], in_=w1[nm].rearrange("(k p) n -> p k n", p=128))
            w1_sb[nm] = t
        ws1_sb = cst.tile([128, 4, OUT], bf16, tag="Ws1")
        nc.sync.dma_start(out=ws1_sb[:], in_=w1["Ws1"].rearrange("(k p) n -> p k n", p=128))
        b_sb = {}
        for nm, width in (("bq0", HC), ("bs0", HC),
                          ("bq1", HC), ("bs1", OUT)):
            t = cst.tile([1, width], bf16, tag=nm)
            nc.sync.dma_start(out=t[:], in_=bias[nm])
            b_sb[nm] = t
        ones = cst.tile([1, 128], bf16, tag="ones")
        nc.gpsimd.memset(ones[:], 1.0)
        ident = cst.tile([128, 128], bf16, tag="ident")
        make_identity(nc, ident[:])


        def proj_tile(xt, W_sb, b_row, m, psum_pool):
            """psum[m, width] = xt.T @ W (+ 1s*b when b_row given).
            k needs no bias (softmax shift-invariance per destination); the v
            bias is folded into the skip table's bias on the host."""
            last = b_row is None
            if len(W_sb.shape) == 3:
                width = W_sb.shape[2]
                ps = psum_pool.tile([TD, width], f32, tag="pp")
                for kk in range(4):
                    nc.tensor.matmul(ps[:m], lhsT=xt[:, kk, :m], rhs=W_sb[:, kk, :],
                                     start=(kk == 0), stop=(last and kk == 3))
            else:
                width = W_sb.shape[1]
                ps = psum_pool.tile([TD, width], f32, tag="pp")
                nc.tensor.matmul(ps[:m], lhsT=xt[:, :m], rhs=W_sb[:], start=True, stop=last)
            if b_row is not None:
                nc.tensor.matmul(ps[:m], lhsT=ones[:, :m], rhs=b_row[:], start=False, stop=True)
            return ps

        # ---- phase P0: L0 projections (kv0 sharded + chunked AllGather) ----
        for rt in range(NT):
            sl = slice(rt * TD, (rt + 1) * TD)
            xt = prj.tile([D_IN, TD], bf16, tag="xt")
            nc.scalar.dma_start(out=xt[:], in_=xTo[:, sl])
            ps_k = proj_tile(xt, w0_sb["Wk0"], None, TD, psP)
            kb = prj.tile([TD, HC], bf16, tag="kb")
            nc.scalar.copy(out=kb[:], in_=ps_k[:])
            nc.sync.dma_start(out=kv0_b[sl, :HC], in_=kb[:])
            ps_v = proj_tile(xt, w0_sb["Wv0"], None, TD, psP)
            vb = prj.tile([TD, HC], bf16, tag="vb")
            nc.scalar.copy(out=vb[:], in_=ps_v[:])
            nc.sync.dma_start(out=kv0_b[sl, HC:], in_=vb[:])
            ps_q = proj_tile(xt, w0_sb["Wq0"], b_sb["bq0"], TD, psP)
            qb = prj.tile([TD, HC], bf16, tag="qb")
            nc.scalar.copy(out=qb[:], in_=ps_q[:])
            nc.sync.dma_start(out=q0_loc[sl, :], in_=qb[:])
            ps_s = proj_tile(xt, w0_sb["Ws0"], b_sb["bs0"], TD, psP)
            sb_ = prj.tile([TD, HC], f32, tag="sb")
            nc.scalar.copy(out=sb_[:], in_=ps_s[:])
            nc.sync.dma_start(out=s0_own[sl, :], in_=sb_[:])
        for c in range(AGC):
            nc.gpsimd.collective_compute(
                "AllGather", mybir.AluOpType.bypass,
                replica_groups=[list(range(NCORES))],
                ins=[kv0_b.ap()[c * CR:(c + 1) * CR, :].opt()],
                outs=[kv0_full.ap()[c * CRN:(c + 1) * CRN, :].opt()])

        def edge_tile(t, q_tab, kv_tab):
            """Group-pipelined gather + logits + masked aggregation for one
            dst-tile; gather group c reads only AG chunk c of the table.
            Returns (psum_agg [TD, HC] f32, psum_den [TD, H] f32)."""
            q_tile = sml.tile([TD, HC], bf16, tag="qtl")
            nc.sync.dma_start(out=q_tile[:], in_=q_tab[t * TD:(t + 1) * TD, :])

            lg = sml.tile([128, GRP * B * H], f32, tag="lg")
            wexp = sml.tile([128, GRP * B * H], bf16, tag="wexp")
            ps_agg = psA.tile([TD, HC], f32, tag="agg")
            ps_den = psD.tile([TD, H], f32, tag="den")

            for c in range(GRP):
                src_t = sml.tile([128, B * 8], i16, tag="src")
                nc.sync.dma_start(out=src_t[:], in_=srcw[t, c])
                kvc = edg.tile([128, B, 2 * HC], bf16, tag="kv")
                for g0 in range(0, B, 8):
                    g1 = min(g0 + 8, B)
                    nc.gpsimd.dma_gather(
                        out_ap=kvc[:, g0:g1, :],
                        in_ap=kv_tab[c * GRN:(c + 1) * GRN, :],
                        idxs_ap=src_t[:, g0 * 8:g1 * 8],
                        num_idxs=(g1 - g0) * 128, num_idxs_reg=(g1 - g0) * 128,
                        elem_size=2 * HC, elem_step=2 * HC)
                sb0 = c * GS
                moh = edg.tile([128, B, 128], fp8, tag="moh")
                nc.sync.dma_start(
                    out=moh[:],
                    in_=mw[t, :, sb0:sb0 + GS].rearrange("p (b d) -> p b d", d=128))
                mtc = edg.tile([TD, GS], fp8, tag="mt")
                nc.scalar.dma_start(out=mtc[:], in_=mtw[t, :, sb0:sb0 + GS])
                qkc = edg.tile([128, B, HC], bf16, tag="qkt")
                for b in range(B):
                    ps_qg = psP.tile([128, 512], f32, tag="pp")
                    nc.tensor.matmul(ps_qg[:, :HC], lhsT=mtc[:, b * 128:(b + 1) * 128],
                                     rhs=q_tile[:], start=True, stop=True)
                    nc.scalar.copy(out=qkc[:, b, :], in_=ps_qg[:, :HC])
                nc.vector.tensor_tensor(out=qkc[:], in0=qkc[:], in1=kvc[:, :, :HC],
                                        op=mybir.AluOpType.mult)
                lgs = lg[:, (c * B) * H:(c * B + B) * H]
                nc.vector.tensor_reduce(
                    out=lgs, in_=qkc[:].rearrange("p b (h c) -> p (b h) c", c=HID),
                    axis=mybir.AxisListType.X, op=mybir.AluOpType.add)
                wes = wexp[:, (c * B) * H:(c * B + B) * H]
                nc.scalar.activation(out=wes, in_=lgs,
                                     func=mybir.ActivationFunctionType.Exp,
                                     scale=float(SCALE))
                vview = kvc[:, :, HC:].rearrange("p b (h c) -> p b h c", c=HID)
                wview = wes.rearrange("p (b h) -> p b h", h=H)
                nc.vector.tensor_tensor(
                    out=vview, in0=vview,
                    in1=wview[:, :, :, None].to_broadcast([128, B, H, HID]),
                    op=mybir.AluOpType.mult)
                for b in range(B):
                    nc.tensor.matmul(ps_agg[:], lhsT=moh[:, b, :TD],
                                     rhs=kvc[:, b, HC:],
                                     start=(c == 0 and b == 0),
                                     stop=(c == GRP - 1 and b == B - 1))
                    nc.tensor.matmul(ps_den[:], lhsT=moh[:, b, :TD],
                                     rhs=wexp[:, (c * B + b) * H:(c * B + b + 1) * H],
                                     start=(c == 0 and b == 0),
                                     stop=(c == GRP - 1 and b == B - 1))
            return ps_agg, ps_den

        # ---- phase P1: L0 edge phase + per-tile L1 projections ----
        for t in range(NT):
            sl = slice(t * TD, (t + 1) * TD)
            ps_agg, ps_den = edge_tile(t, q0_loc.ap(), kv0_full.ap())
            den = sml.tile([TD, H], f32, tag="den_s")
            nc.vector.tensor_scalar(out=den[:], in0=ps_den[:], scalar1=1e-16,
                                    scalar2=None, op0=mybir.AluOpType.add)
            rec = sml.tile([TD, H], f32, tag="rec")
            nc.vector.reciprocal(out=rec[:], in_=den[:])
            s0_t = sml.tile([TD, HC], f32, tag="s0t")
            nc.sync.dma_start(out=s0_t[:], in_=s0_own[sl, :])
            hn = sml.tile([TD, HC], f32, tag="hn")
            nc.vector.tensor_tensor(
                out=hn[:].rearrange("p (h c) -> p h c", h=H),
                in0=ps_agg[:].rearrange("p (h c) -> p h c", h=H),
                in1=rec[:, :, None].to_broadcast([TD, H, HID]),
                op=mybir.AluOpType.mult)
            nc.vector.tensor_tensor(out=hn[:], in0=hn[:], in1=s0_t[:],
                                    op=mybir.AluOpType.add)
            h_bf = sml.tile([TD, HC], bf16, tag="hbf")
            nc.scalar.activation(out=h_bf[:], in_=hn[:],
                                 func=mybir.ActivationFunctionType.Relu)

            # transpose h tile -> hT [128, 4, TD]
            hT = sml.tile([128, 4, TD], bf16, tag="hT")
            for kk in range(4):
                ps_t = psT.tile([128, TD], bf16, tag="tr")
                nc.tensor.transpose(ps_t[:], in_=h_bf[:, kk * 128:(kk + 1) * 128],
                                    identity=ident[:TD, :TD])
                nc.scalar.copy(out=hT[:, kk, :], in_=ps_t[:])

            # L1 projections for this tile
            ps_q1 = proj_tile(hT, w1_sb["Wq1"], b_sb["bq1"], TD, psP)
            q1b = prj.tile([TD, HC], bf16, tag="q1b")
            nc.scalar.copy(out=q1b[:], in_=ps_q1[:])
            nc.sync.dma_start(out=q1_loc[sl, :], in_=q1b[:])
            ps_k1 = proj_tile(hT, w1_sb["Wk1"], None, TD, psP)
            k1b = prj.tile([TD, HC], bf16, tag="k1b")
            nc.scalar.copy(out=k1b[:], in_=ps_k1[:])
            nc.sync.dma_start(out=kv1_b[sl, :HC], in_=k1b[:])
            ps_v1 = proj_tile(hT, w1_sb["Wv1"], None, TD, psP)
            v1b = prj.tile([TD, HC], bf16, tag="v1b")
            nc.scalar.copy(out=v1b[:], in_=ps_v1[:])
            nc.scalar.dma_start(out=kv1_b[sl, HC:], in_=v1b[:])
            ps_s1 = proj_tile(hT, ws1_sb, b_sb["bs1"], TD, psP)
            s1b = prj.tile([TD, OUT], f32, tag="s1b")
            nc.scalar.copy(out=s1b[:], in_=ps_s1[:])
            nc.sync.dma_start(out=s1_own[sl, :], in_=s1b[:])

        # ---- AllGather [k1|v1], chunked for overlap with the P1 tail ----
        # Table rows are permuted [chunk][core][row] (see _perm_rows) so each
        # chunk's AllGather output is a contiguous slice.
        rows = NPC // AGC
        for c in range(AGC):
            nc.gpsimd.collective_compute(
                "AllGather", mybir.AluOpType.bypass,
                replica_groups=[list(range(NCORES))],
                ins=[kv1_b.ap()[c * rows:(c + 1) * rows, :].opt()],
                outs=[kv1_full.ap()[c * rows * NCORES:(c + 1) * rows * NCORES, :].opt()])

        # ---- phase P2: L1 edge phase ----
        for t in range(NT):
            sl = slice(t * TD, (t + 1) * TD)
            ps_agg, ps_den = edge_tile(t, q1_loc.ap(), kv1_full.ap())
            den = sml.tile([TD, H], f32, tag="den_s")
            nc.vector.tensor_scalar(out=den[:], in0=ps_den[:], scalar1=float(H),
                                    scalar2=1e-16, op0=mybir.AluOpType.mult,
                                    op1=mybir.AluOpType.add)
            rec = sml.tile([TD, H], f32, tag="rec")
            nc.vector.reciprocal(out=rec[:], in_=den[:])
            hn = sml.tile([TD, HC], f32, tag="hn")
            nc.vector.tensor_tensor(
                out=hn[:].rearrange("p (h c) -> p h c", h=H),
                in0=ps_agg[:].rearrange("p (h c) -> p h c", h=H),
                in1=rec[:, :, None].to_broadcast([TD, H, HID]),
                op=mybir.AluOpType.mult)
            hm = sml.tile([TD, OUT], f32, tag="hm")
            nc.vector.tensor_reduce(out=hm[:], in_=hn[:].rearrange("p (h c) -> p c h", h=H),
                                    axis=mybir.AxisListType.X,
                                    op=mybir.AluOpType.add)
            s1_t = sml.tile([TD, OUT], f32, tag="s1t")
            nc.sync.dma_start(out=s1_t[:], in_=s1_own[sl, :])
            ot = sml.tile([TD, OUT], f32, tag="ot")
            nc.vector.tensor_tensor(out=ot[:], in0=hm[:], in1=s1_t[:],
                                    op=mybir.AluOpType.add)
            nc.sync.dma_start(out=out[sl, :], in_=ot[:])

    nc.compile()
    return nc


def _prep_edges(src, dst):
    """Sort edges by (dst-tile, src-AG-chunk); build per-(core,tile,group)
    padded slot arrays with chunk-local permuted gather indices."""
    ds = dst.astype(np.int64)
    ss = src.astype(np.int64)
    sp = _perm_row(ss)                      # permuted table row of src
    tid = ds // TD                          # global dst tile 0..159
    grp = sp // (N // GRP)                  # gather group of src
    n_tiles = NCORES * NT
    key = tid * GRP + grp
    order = np.argsort(key, kind="stable")
    key_s = key[order]
    sp_s = sp[order]
    ds_s = ds[order]
    cnt = np.bincount(key_s, minlength=n_tiles * GRP)
    B = int(np.ceil(cnt.max() / 128))
    GS = B * 128
    offs = np.zeros(n_tiles * GRP + 1, np.int64)
    np.cumsum(cnt, out=offs[1:])
    pos = np.arange(E) - offs[key_s]

    src_pad = np.zeros((n_tiles * GRP, GS), np.int16)
    dstc_pad = np.full((n_tiles * GRP, GS), -1.0, np.float32)
    src_pad[key_s, pos] = sp_s % (N // GRP)         # group-local row
    dstc_pad[key_s, pos] = (ds_s - (key_s // GRP) * TD).astype(np.float32)

    # 16-wrapped gather index layout, replicated across the 8 Q7 cores
    w = src_pad.reshape(n_tiles * GRP, GS // 16, 16).transpose(0, 2, 1)
    w = np.tile(w, (1, 8, 1))
    srcw = np.ascontiguousarray(w).reshape(NCORES, NT, GRP, 128, GS // 16)

    FP8 = ml_dtypes.float8_e4m3
    SLOTS = GRP * GS
    dtile = dstc_pad.reshape(n_tiles, SLOTS)
    dr = dtile.reshape(n_tiles, SLOTS // 128, 128)
    m1 = (dr[:, :, :, None] == np.arange(128, dtype=np.float32))
    mw = np.ascontiguousarray(m1.transpose(0, 2, 1, 3)).astype(FP8).reshape(
        NCORES, NT, 128, SLOTS)
    mtw = (dtile[:, None, :] == np.arange(TD, dtype=np.float32)[None, :, None]
           ).astype(FP8).reshape(NCORES, NT, TD, SLOTS)
    return B, srcw, mw, mtw


def kernel(x, edge_index,
           Wq0, bq0, Wk0, bk0, Wv0, bv0, Ws0, bs0,
           Wq1, bq1, Wk1, bk1, Wv1, bv1, Ws1, bs1):
    global LAST_EXEC_NS, LAST_RESULTS
    _install_ntff_hook()
    from concourse import bass_utils

    x = np.asarray(x, np.float32)
    edge_index = np.asarray(edge_index)
    B, srcw, mw, mtw = _prep_edges(edge_index[0], edge_index[1])

    if B not in _NC_CACHE:
        _NC_CACHE[B] = _build_nc(B)
    nc = _NC_CACHE[B]

    xT = np.ascontiguousarray(np.asarray(x).T).astype(BF16)
    shared = dict(xT=xT)
    for nm, a in (("Wq0", Wq0), ("Wk0", Wk0), ("Wv0", Wv0), ("Ws0", Ws0),
                  ("Wq1", Wq1), ("Wk1", Wk1), ("Wv1", Wv1), ("Ws1", Ws1)):
        shared[nm] = np.asarray(a).astype(BF16)
    # k biases are softmax-invariant (dropped); v biases fold into the skip
    # tables: h = relu(num/den + bv0 + x@Ws0 + bs0), L1 mean-over-heads of bv1.
    bs0f = np.asarray(bs0, np.float32) + np.asarray(bv0, np.float32)
    bs1f = (np.asarray(bs1, np.float32)
            + np.asarray(bv1, np.float32).reshape(H, OUT).mean(axis=0))
    for nm, a in (("bq0", bq0), ("bs0", bs0f),
                  ("bq1", bq1), ("bs1", bs1f)):
        shared[nm] = np.asarray(a).astype(BF16)[None, :]

    in_maps = []
    for c in range(NCORES):
        m = dict(shared)
        m["xTo"] = np.ascontiguousarray(xT[:, c * NPC:(c + 1) * NPC])
        m["srcw"] = srcw[c]
        m["mw"] = mw[c]
        m["mtw"] = mtw[c]
        in_maps.append(m)

    res = bass_utils.run_bass_kernel_spmd(
        nc, in_maps, core_ids=list(range(NCORES)), trace=TRACE)
    LAST_EXEC_NS = res.exec_time_ns
    LAST_RESULTS = res
    out = np.concatenate([res.results[c]["out"] for c in range(NCORES)], axis=0)
    return np.ascontiguousarray(out, dtype=np.float32)
